# revision 1
# baseline (speedup 1.0000x reference)
"""BoxFuse (sparse_attention) Trainium2 Bass kernel.

Data-parallel over batch: 32 batches -> 8 NeuronCores x 4 batches.
Per core, for each batch b:
  q = LN(vit) @ qw' + qb'      (qw' has LN gamma + attention scale folded in)
  k = LN(box) @ kw' + kb'
  v = LN(box) @ vw' + vb'
  attT[l, n] = sum_d kT[d, l] * qT[d, n]          (computed transposed)
  w = exp(attT * mask_scale + mask_bias)           (masked, no max-subtraction)
  out[n, :] = (w[:, n] . v) / sum_l(w[l, n]) + vit[n, :]

Matmuls run in float32r (TF32-like, 1 cyc/row at free-dim >= 256).
Layout notes: all projections produce the *transposed* activations directly
(lhsT = weights, rhs = x_hatT), so the only transposes are of the LN'd
inputs (PE transpose-mode via identity) and of vT back to v-natural.
"""

import os
import numpy as np

# The Bass kernel executes via the axon/neuron jax platform; a stray
# JAX_PLATFORMS=cpu (e.g. set for running the jax reference) would hide the
# NeuronCores from the runtime.
if os.environ.get("JAX_PLATFORMS", "").strip() == "cpu":
    os.environ.pop("JAX_PLATFORMS")

B, NTOK, L, LOW, HIGH = 32, 576, 100, 1024, 1536
NCORES = 8
BPC = B // NCORES            # batches per core
LN_EPS = 1e-5
MASK_NEG = -30.0
HT_LOW = LOW // 128          # 8 h-tiles for vit
HT_HIGH = HIGH // 128        # 12 h-tiles for box
DT = LOW // 128              # 8 d-tiles of output features
LB = BPC * L                 # 400: batch-concat box token dim

_CACHE = {}


def _mm_dtypes():
    import concourse.mybir as mybir
    use_f32 = os.environ.get("BOXFUSE_MM_DT", "f32r") == "f32"
    return (mybir.dt.float32 if use_f32 else mybir.dt.float32r), mybir.dt.float32


def _build(reps=1):
    import concourse.bacc as bacc
    import concourse.tile as tile
    import concourse.mybir as mybir

    STAGE = os.environ.get("BOXFUSE_STAGE", "full")
    MMDT, F32 = _mm_dtypes()
    TRDT = MMDT if os.environ.get("BF_TR", "") == "r" else F32
    AF = mybir.ActivationFunctionType
    ALU = mybir.AluOpType

    nc = bacc.Bacc("TRN2", target_bir_lowering=False, debug=False)

    vit_d = nc.dram_tensor("vit", [BPC, NTOK, LOW], F32, kind="ExternalInput").ap()
    box_d = nc.dram_tensor("box", [BPC, L, HIGH], F32, kind="ExternalInput").ap()
    qw_d = nc.dram_tensor("qw", [LOW, LOW], MMDT, kind="ExternalInput").ap()
    kw_d = nc.dram_tensor("kw", [HIGH, LOW], MMDT, kind="ExternalInput").ap()
    vw_d = nc.dram_tensor("vw", [HIGH, LOW], MMDT, kind="ExternalInput").ap()
    qb_d = nc.dram_tensor("qb", [LOW], F32, kind="ExternalInput").ap()
    kb_d = nc.dram_tensor("kb", [LOW], F32, kind="ExternalInput").ap()
    vb_d = nc.dram_tensor("vb", [LOW], F32, kind="ExternalInput").ap()
    msc_d = nc.dram_tensor("msc", [L, BPC], F32, kind="ExternalInput").ap()
    mbs_d = nc.dram_tensor("mbs", [L, BPC], F32, kind="ExternalInput").ap()
    ident_d = nc.dram_tensor("ident", [128, 128], TRDT, kind="ExternalInput").ap()
    ones_d = nc.dram_tensor("ones", [128, 1], F32, kind="ExternalInput").ap()
    out_d = nc.dram_tensor("out", [BPC, NTOK, LOW], F32, kind="ExternalOutput").ap()

    with tile.TileContext(nc) as tc:
      for _rep in range(reps):
        with (
            tc.tile_pool(name="consts", bufs=1) as consts,
            tc.tile_pool(name="persist", bufs=1) as persist,
            tc.tile_pool(name="small", bufs=int(os.environ.get("BF_SM", "6"))) as small,
            tc.tile_pool(name="pp_t", bufs=int(os.environ.get("BF_PPT", "3")), space="PSUM") as pp_t,
            tc.tile_pool(name="pp_proj", bufs=int(os.environ.get("BF_PJ", "2")), space="PSUM") as pp_proj,
            tc.tile_pool(name="pp_att", bufs=int(os.environ.get("BF_ATT", "3")), space="PSUM") as pp_att,
        ):
            ident = consts.tile([128, 128], TRDT, tag="ident")
            nc.sync.dma_start(ident[:], ident_d)
            ones = consts.tile([128, 1], F32, tag="ones")
            nc.sync.dma_start(ones[:], ones_d)
            msc = consts.tile([128, BPC], F32, tag="msc")
            nc.sync.dma_start(msc[:L, :], msc_d)
            mbs = consts.tile([128, BPC], F32, tag="mbs")
            nc.sync.dma_start(mbs[:L, :], mbs_d)
            qb = consts.tile([128, DT], F32, tag="qb")
            nc.sync.dma_start(qb[:], qb_d.rearrange("(t p) -> p t", p=128))
            kb = consts.tile([128, DT], F32, tag="kb")
            nc.sync.dma_start(kb[:], kb_d.rearrange("(t p) -> p t", p=128))
            vb = consts.tile([128, DT], F32, tag="vb")
            nc.sync.dma_start(vb[:], vb_d.rearrange("(t p) -> p t", p=128))

            eps_t = consts.tile([128, 1], F32, tag="eps")
            nc.vector.memset(eps_t[:], LN_EPS)

            kT = persist.tile([128, DT, LB], MMDT, tag="kT")    # k^T[d, l-concat]
            vnat = persist.tile([128, BPC, LOW], MMDT, tag="v")  # v[l, d] per batch

            def layernorm_stats(x_ap, rows, width):
                """x_ap: [rows, width] fp32 in SBUF -> (r, neg_mean_r) [rows,1]."""
                chunks = width // 512
                st6 = small.tile([128, chunks, 6], F32, tag="st6")
                for c in range(chunks):
                    nc.vector.bn_stats(
                        st6[:rows, c, :], x_ap[:rows, c * 512:(c + 1) * 512]
                    )
                st2 = small.tile([128, 2], F32, tag="st2")
                nc.vector.bn_aggr(st2[:rows, :], st6[:rows, :, :])
                sd = small.tile([128, 1], F32, tag="sd")
                # sqrt(var + eps)
                nc.scalar.activation(sd[:rows, :], st2[:rows, 1:2], AF.Sqrt,
                                     bias=eps_t[:rows, :], scale=1.0)
                r = small.tile([128, 1], F32, tag="r")
                nc.vector.reciprocal(r[:rows, :], sd[:rows, :])
                nmr = small.tile([128, 1], F32, tag="nmr")
                # nmr = (mean * -1) * r
                nc.vector.scalar_tensor_tensor(
                    nmr[:rows, :], st2[:rows, 0:1], -1.0, r[:rows, :],
                    op0=ALU.mult, op1=ALU.mult,
                )
                return r, nmr

            # ---------------- Phase A: box -> boxT, k^T, v ----------------
            with (
                tc.tile_pool(name="wA", bufs=2) as wA,
                tc.tile_pool(name="stageA", bufs=int(os.environ.get("BF_SA", "2"))) as stageA,
                tc.tile_pool(name="boxTp", bufs=1) as boxTp,
                tc.tile_pool(name="vTp", bufs=1) as vTp,
            ):
                boxT = boxTp.tile([128, HT_HIGH, LB], MMDT, tag="boxT")
                for b in range(BPC):
                    bx = stageA.tile([128, HIGH], F32, tag="bx")
                    nc.sync.dma_start(bx[:L, :], box_d[b])
                    r, nmr = layernorm_stats(bx, L, HIGH)
                    xh = stageA.tile([128, HIGH], TRDT, tag="xhb")
                    nc.scalar.activation(xh[:L, :], bx[:L, :], AF.Identity,
                                         bias=nmr[:L, :], scale=r[:L, :])
                    for h in range(HT_HIGH):
                        tp = pp_t.tile([128, 128], TRDT, tag="tp")
                        nc.tensor.transpose(
                            tp[:, :L], xh[:L, h * 128:(h + 1) * 128], ident[:L, :L]
                        )
                        nc.vector.tensor_copy(
                            boxT[:, h, b * L:(b + 1) * L], tp[:, :L]
                        )

                # k^T = kw^T @ box_lnT   [d, l-concat]
                kw = wA.tile([128, HT_HIGH, LOW], MMDT, tag="wA")
                for h in range(HT_HIGH):
                    nc.sync.dma_start(kw[:, h, :], kw_d[h * 128:(h + 1) * 128, :])
                for d in range(DT):
                    ps = pp_proj.tile([128, 512], F32, tag="pj")
                    for h in range(HT_HIGH):
                        nc.tensor.matmul(
                            ps[:, :LB], kw[:, h, d * 128:(d + 1) * 128],
                            boxT[:, h, :], start=(h == 0), stop=(h == HT_HIGH - 1),
                        )
                    nc.scalar.activation(kT[:, d, :], ps[:, :LB], AF.Identity,
                                         bias=kb[:, d:d + 1], scale=1.0)

                # v^T then transpose to v-natural
                vT = vTp.tile([128, DT, LB], TRDT, tag="vT")
                vw = wA.tile([128, HT_HIGH, LOW], MMDT, tag="wA")
                for h in range(HT_HIGH):
                    nc.sync.dma_start(vw[:, h, :], vw_d[h * 128:(h + 1) * 128, :])
                for d in range(DT):
                    ps = pp_proj.tile([128, 512], F32, tag="pj")
                    for h in range(HT_HIGH):
                        nc.tensor.matmul(
                            ps[:, :LB], vw[:, h, d * 128:(d + 1) * 128],
                            boxT[:, h, :], start=(h == 0), stop=(h == HT_HIGH - 1),
                        )
                    nc.scalar.activation(vT[:, d, :], ps[:, :LB], AF.Identity,
                                         bias=vb[:, d:d + 1], scale=1.0)
                for d in range(DT):
                    for b in range(BPC):
                        tp = pp_t.tile([128, 128], TRDT, tag="tp")
                        nc.tensor.transpose(
                            tp[:L, :], vT[:, d, b * L:(b + 1) * L], ident[:, :]
                        )
                        nc.vector.tensor_copy(
                            vnat[:L, b, d * 128:(d + 1) * 128], tp[:L, :]
                        )

            if STAGE == "A":
                dbg = small.tile([128, 128], F32, tag="dbg")
                nc.vector.tensor_copy(dbg[:], kT[:, 0, :128].bitcast(F32))
                nc.sync.dma_start(out_d[0, 0:128, 0:128], dbg[:])
                dbg2 = small.tile([128, 128], F32, tag="dbg")
                nc.vector.tensor_copy(dbg2[:], vnat[:, 0, :128].bitcast(F32))
                nc.sync.dma_start(out_d[0, 128:256, 0:128], dbg2[:])
            # ---------------- Phase B: per batch ----------------
            with (
                tc.tile_pool(name="qwp", bufs=1) as qwp,
                tc.tile_pool(name="vitp", bufs=int(os.environ.get("BF_VIT", "2"))) as vitp,
                tc.tile_pool(name="xTp", bufs=int(os.environ.get("BF_XT", "1"))) as xTp,
                tc.tile_pool(name="qTp", bufs=int(os.environ.get("BF_QT", "1"))) as qTp,
                tc.tile_pool(name="attp", bufs=int(os.environ.get("BF_ATTP", "2"))) as attp,
                tc.tile_pool(name="stageB", bufs=int(os.environ.get("BF_SB", "3"))) as stageB,
                tc.tile_pool(name="outp", bufs=int(os.environ.get("BF_OUT", "3"))) as outp,
            ):
                qw = qwp.tile([128, HT_LOW, LOW], MMDT, tag="qw")
                for h in range(HT_LOW):
                    nc.sync.dma_start(qw[:, h, :], qw_d[h * 128:(h + 1) * 128, :])

                NT = [(t * 128, min(128, NTOK - t * 128)) for t in range(5)]

                for b in range(0 if STAGE == "A" else BPC):
                    vit_nat = vitp.tile([128, 5, LOW], F32, tag="vit")
                    for t, (st, w) in enumerate(NT):
                        nc.sync.dma_start(vit_nat[:w, t, :], vit_d[b, st:st + w, :])

                    xT = xTp.tile([128, HT_LOW, NTOK], MMDT, tag="xT")
                    for t, (st, w) in enumerate(NT):
                        r, nmr = layernorm_stats(vit_nat[:, t, :], w, LOW)
                        xh = stageB.tile([128, LOW], TRDT, tag="xhv")
                        nc.scalar.activation(xh[:w, :], vit_nat[:w, t, :],
                                             AF.Identity, bias=nmr[:w, :],
                                             scale=r[:w, :])
                        for h in range(HT_LOW):
                            tp = pp_t.tile([128, 128], TRDT, tag="tp")
                            nc.tensor.transpose(
                                tp[:, :w], xh[:w, h * 128:(h + 1) * 128],
                                ident[:w, :w],
                            )
                            nc.vector.tensor_copy(xT[:, h, st:st + w], tp[:, :w])

                    if STAGE == "T":
                        dbg = small.tile([128, 128], F32, tag="dbg")
                        nc.vector.tensor_copy(dbg[:], xT[:, 0, :128].bitcast(F32))
                        nc.sync.dma_start(out_d[b, 0:128, 0:128], dbg[:])
                        continue
                    # q^T = qw^T @ x_hatT  [d, n], chunks of 288
                    qT = qTp.tile([128, HT_LOW, NTOK], MMDT, tag="qT")
                    for d in range(DT):
                        for c in range(2):
                            cs = c * 288
                            ps = pp_proj.tile([128, 512], F32, tag="pj")
                            for h in range(HT_LOW):
                                nc.tensor.matmul(
                                    ps[:, :288], qw[:, h, d * 128:(d + 1) * 128],
                                    xT[:, h, cs:cs + 288],
                                    start=(h == 0), stop=(h == HT_LOW - 1),
                                )
                            nc.scalar.activation(qT[:, d, cs:cs + 288], ps[:, :288],
                                                 AF.Identity, bias=qb[:, d:d + 1],
                                                 scale=1.0)

                    if STAGE == "Q":
                        dbg = small.tile([128, 128], F32, tag="dbg")
                        nc.vector.tensor_copy(dbg[:], qT[:, 0, :128].bitcast(F32))
                        nc.sync.dma_start(out_d[b, 0:128, 0:128], dbg[:])
                        continue
                    # attT[l, n] = k^T . q^T over d; exp with mask fused
                    attT = attp.tile([128, NTOK], MMDT, tag="attT")
                    for c in range(2):
                        cs = c * 288
                        ps = pp_att.tile([128, 512], F32, tag="att")
                        for d in range(DT):
                            nc.tensor.matmul(
                                ps[:L, :288], kT[:, d, b * L:(b + 1) * L],
                                qT[:, d, cs:cs + 288],
                                start=(d == 0), stop=(d == DT - 1),
                            )
                        nc.scalar.activation(attT[:L, cs:cs + 288], ps[:L, :288],
                                             AF.Exp, bias=mbs[:L, b:b + 1],
                                             scale=msc[:L, b:b + 1])

                    if STAGE == "K":
                        dbg = small.tile([128, 576], F32, tag="dbgw")
                        nc.vector.tensor_copy(dbg[:L, :], attT[:L, :].bitcast(F32))
                        nc.sync.dma_start(out_d[b, 0:100, 0:576], dbg[:L, :])
                        continue
                    # per n-slice: sums, 1/sum, att@v, epilogue
                    inv = small.tile([128, 5], F32, tag="inv")
                    for s, (st, w) in enumerate(NT):
                        pss = pp_att.tile([128, 512], F32, tag="att")
                        nc.tensor.matmul(pss[:w, :1],
                                         attT[:L, st:st + w].bitcast(F32),
                                         ones[:L, :], start=True, stop=True)
                        nc.vector.reciprocal(inv[:w, s:s + 1], pss[:w, :1])
                        outst = outp.tile([128, LOW], F32, tag="outst")
                        for c in range(2):
                            cs = c * 512
                            psv = pp_att.tile([128, 512], F32, tag="att")
                            nc.tensor.matmul(
                                psv[:w, :], attT[:L, st:st + w],
                                vnat[:L, b, cs:cs + 512], start=True, stop=True,
                            )
                            # out = psv * inv + vit
                            nc.vector.scalar_tensor_tensor(
                                outst[:w, cs:cs + 512], psv[:w, :],
                                inv[:w, s:s + 1], vit_nat[:w, s, cs:cs + 512],
                                op0=ALU.mult, op1=ALU.add,
                            )
                        nc.sync.dma_start(out_d[b, st:st + w, :], outst[:w, :])

    nc.compile()
    return nc


def kernel(**inputs):
    from concourse.bass_utils import run_bass_kernel_spmd

    vit = np.ascontiguousarray(inputs["vit_feat"], dtype=np.float32)
    box = np.ascontiguousarray(inputs["box_feat"], dtype=np.float32)
    lengths = np.asarray(inputs["lengths"])
    f32 = np.float32

    def eff(ln_w, ln_b, w, bias, scale=1.0):
        w = np.asarray(w, f32)
        weff = (np.asarray(ln_w, f32)[:, None] * w) * f32(scale)
        beff = (np.asarray(ln_b, f32) @ w + np.asarray(bias, f32)) * f32(scale)
        return np.ascontiguousarray(weff), np.ascontiguousarray(beff)

    att_scale = 1.0 / np.sqrt(np.float32(LOW))
    qw, qb = eff(inputs["q_ln_w"], inputs["q_ln_b"], inputs["q_w"], inputs["q_b"],
                 att_scale)
    kw, kb = eff(inputs["k_ln_w"], inputs["k_ln_b"], inputs["k_w"], inputs["k_b"])
    vw, vb = eff(inputs["v_ln_w"], inputs["v_ln_b"], inputs["v_w"], inputs["v_b"])

    valid = (np.arange(L)[None, :] < lengths[:, None].astype(np.int64))  # [B, L]
    msc_all = valid.astype(f32)                       # 1 valid / 0 masked
    mbs_all = np.where(valid, f32(0.0), f32(MASK_NEG))
    ident = np.eye(128, dtype=f32)
    ones = np.ones((128, 1), dtype=f32)

    if "nc" not in _CACHE:
        _CACHE["nc"] = _build()
    nc = _CACHE["nc"]

    in_maps = []
    for c in range(NCORES):
        sl = slice(c * BPC, (c + 1) * BPC)
        in_maps.append({
            "vit": vit[sl], "box": box[sl],
            "qw": qw, "kw": kw, "vw": vw,
            "qb": qb, "kb": kb, "vb": vb,
            "msc": np.ascontiguousarray(msc_all[sl].T),
            "mbs": np.ascontiguousarray(mbs_all[sl].T),
            "ident": ident, "ones": ones,
        })

    _CACHE["in_maps"] = in_maps
    res = run_bass_kernel_spmd(nc, in_maps, core_ids=list(range(NCORES)))
    out = np.concatenate([res.results[c]["out"] for c in range(NCORES)], axis=0)
    return np.ascontiguousarray(out.astype(np.float32))


if __name__ == "__main__":
    rng = np.random.default_rng(0)
    ins = {
        "vit_feat": rng.standard_normal((B, NTOK, LOW)).astype(np.float32),
        "box_feat": rng.standard_normal((B, L, HIGH)).astype(np.float32),
        "lengths": rng.integers(0, L, (B,)).astype(np.int64),
        "q_ln_w": np.ones(LOW, np.float32), "q_ln_b": np.zeros(LOW, np.float32),
        "q_w": (rng.standard_normal((LOW, LOW)) * 0.02).astype(np.float32),
        "q_b": np.zeros(LOW, np.float32),
        "k_ln_w": np.ones(HIGH, np.float32), "k_ln_b": np.zeros(HIGH, np.float32),
        "k_w": (rng.standard_normal((HIGH, LOW)) * 0.02).astype(np.float32),
        "k_b": np.zeros(LOW, np.float32),
        "v_ln_w": np.ones(HIGH, np.float32), "v_ln_b": np.zeros(HIGH, np.float32),
        "v_w": (rng.standard_normal((HIGH, LOW)) * 0.02).astype(np.float32),
        "v_b": np.zeros(LOW, np.float32),
    }
    out = kernel(**ins)
    print("out", out.shape, out.dtype, np.abs(out).mean())

